# revision 34
# baseline (speedup 1.0000x reference)
"""Bipolar morphological conv2d kernel for Trainium2 (8 NeuronCores).

Math: reference computes, per output position and out-channel c,
    y = m(lp1,K1) - m(lp1,K2) - m(lp2,K1) + m(lp2,K2) + bias
with m(logp, k)[c] = exp(max_p(logp_p + k_pc)), lp1 = log(max(x, .1)),
lp2 = log(max(-x, .1)).

Device algorithm (data-parallel, one batch image per core): the 288-tap
max-plus reduction is evaluated as a tight LSE (p-norm, t=112) over each
3x1 column group of the 3x3 window (96 entries: 3 rows x 32 channels),
turning the heavy reduction into THREE K=96 TensorE matmuls, followed by
an EXACT max over the 3 column groups in log domain (DVE fp16).  A
per-rhs-column normalizer M3q (itself a sigma=32 LSE, computed with a
ones-matmul) keeps every exponential in fp32 range; its value cancels
exactly in the algebra, so only over/underflow matters, not its accuracy.

Layout: channel-major pixel rows.  xT3 [96, 1024] holds the 3 row-shifted
copies of xT [32, 1024] (partition 32g+ci = channel ci shifted g rows),
so a column-group matmul contracts all 96 entries in one instruction and
tap shifts become free-dim column offsets (multiples of 1).  Out-channels
of K1|K2 are stacked on the 128 output partitions, so one matmul chain
serves both kernels.  Final exp folds the per-channel max-k and the
30x30 window selection; a pair of +/-I matmuls transposes to
position-major and combines the 4 morphs with their signs.
"""

import os
from contextlib import ExitStack

import numpy as np

import concourse.bass as bass
import concourse.mybir as mybir
from concourse import bacc
import concourse.tile as tile
from concourse.bass_utils import run_bass_kernel_spmd
from concourse.hw_specs import get_activation_tables
import bass_rust as _bass_rust


class _BaccOneActSet(bacc.Bacc):
    """Bacc whose act-table pass may only pick natural_log_exp_and_others
    (contains Ln+Exp+Copy, the only functions used here), so the table is
    loaded once instead of thrashing on every Ln<->Exp transition."""

    def insert_act_table_loads(self):
        has_activation = any(
            isinstance(i, mybir.InstActivation)
            for b in self.main_func.blocks
            for i in b.instructions
        )
        if not has_activation:
            return
        tables = [
            (n, (fns if n == "natural_log_exp_and_others" else set()))
            for n, fns in get_activation_tables(self.m.arch).items()
        ]
        _bass_rust.insert_act_table_loads(self, tables)

N_CORES = 8
H = W = C = 32
COUT = 64
HO = WO = 30
NPIX = H * W            # 1024
XW = 1024               # working row width (pixels)
XIN = 1088              # input row width (1024 + 64 pad for row shifts)
ACW = 960               # accumulator width (30 rows x 32 cols)
POSW = 958              # last used pos col is 29*32+29 = 957
CW = 964                # chain compute width (cols actually consumed + pad)

SIG1 = 8.0              # stage-1 normalizer LSE sharpness (fits ACT Ln window)
SIG2 = 32.0             # stage-2 normalizer refinement sharpness
T = 112.0               # main LSE sharpness
G = 6.0                 # fixed global scale bound (|x| < 6 for N(0,1) data)
LG = float(np.log(G))
CSH = 216.0             # fp16 recentering shift for the log-domain combine
GCAP = 38.5             # cap on per-channel ln-rescale (ACT Ln window ~|44|)

F32 = mybir.dt.float32
F16 = mybir.dt.float16
BF16 = mybir.dt.bfloat16
_cache: dict = {}
last_results = None


def _ensure_axon_ntff_hook():
    """The trimmed agent image lacks antenv.axon_hooks; recreate it so
    run_bass_kernel_spmd(trace=True) can capture NTFF profiles. No-op on
    failure (tracing then just degrades)."""
    import sys
    import types

    try:
        import antenv.axon_hooks  # noqa: F401
        return
    except ImportError:
        pass
    try:
        mod = types.ModuleType("antenv.axon_hooks")
        holder = [None]
        mod.set_axon_ntff_profile_hook = lambda h: holder.__setitem__(0, h)
        mod.get_axon_ntff_profile_hook = lambda: holder[0]
        sys.modules["antenv.axon_hooks"] = mod
        from trn_agent_boot.trn_boot import _ntff_profile_via_ctypes

        so = "/opt/axon/libaxon_pjrt.so"
        if os.path.exists(so):
            holder[0] = _ntff_profile_via_ctypes(so)
    except Exception:
        pass


def _build_module():
    nc = _BaccOneActSet()
    Alu = mybir.AluOpType
    Act = mybir.ActivationFunctionType

    xT = nc.dram_tensor("xT", [C, XIN], F32, kind="ExternalInput")
    K3 = nc.dram_tensor("K3", [96, 384], BF16, kind="ExternalInput")
    MM = nc.dram_tensor("MM", [128, 128], F16, kind="ExternalInput")
    CM = nc.dram_tensor("CM", [128, 3 + 512], F32, kind="ExternalInput")
    Y = nc.dram_tensor("Y", [128, 512], F32, kind="ExternalOutput")

    with tile.TileContext(nc) as tc, ExitStack() as ctx:
        const = ctx.enter_context(tc.tile_pool(name="const", bufs=1))
        sp = ctx.enter_context(tc.tile_pool(name="sp", bufs=1))
        losb = ctx.enter_context(tc.tile_pool(name="losb", bufs=3))
        accp = ctx.enter_context(tc.tile_pool(name="accp", bufs=1))
        vsb = ctx.enter_context(tc.tile_pool(name="vsb", bufs=1))
        msb = ctx.enter_context(tc.tile_pool(name="msb", bufs=1))
        ysb = ctx.enter_context(tc.tile_pool(name="ysb", bufs=2))
        psS = ctx.enter_context(tc.tile_pool(name="psS", bufs=2, space="PSUM"))
        psO = ctx.enter_context(tc.tile_pool(name="psO", bufs=2, space="PSUM"))

        # ---- constants / input staging ----
        xT3 = const.tile([96, XW], F32)
        for g, eng in ((0, nc.gpsimd), (1, nc.sync), (2, nc.sync)):
            eng.dma_start(out=xT3[32 * g : 32 * g + 32, :],
                          in_=xT[:, 32 * g : 32 * g + XW])
        K3_sb = const.tile([96, 384], BF16)
        nc.sync.dma_start(out=K3_sb[:, :], in_=K3[:, :])
        MM_sb = const.tile([128, 128], F16)
        nc.gpsimd.dma_start(out=MM_sb[:, :], in_=MM[:, :])
        CM_sb = const.tile([128, 3 + 512], F32)
        nc.sync.dma_start(out=CM_sb[:, :], in_=CM[:, :])
        EXM_sb, SC_sb, AFL_sb = CM_sb[:, 0:1], CM_sb[:, 1:2], CM_sb[:, 2:3]
        BCW_sb = CM_sb[:, 3 : 3 + 512]
        ONES_sb = const.tile([96, 128], BF16)
        nc.vector.memset(ONES_sb[:, :], 1.0)
        B8_sb = const.tile([96, 1], F32)
        nc.vector.memset(B8_sb[:, :], -SIG1 * LG)
        B32_sb = const.tile([96, 1], F32)
        nc.vector.memset(B32_sb[:, :], -SIG2 * LG)
        B96_sb = const.tile([96, 1], F32)
        nc.vector.memset(B96_sb[:, :], -T * LG)

        # PE warm-up: junk matmuls bridge the startup window so HAM
        # reaches K=8/8 before the first real matmul (and stays there).
        Wm = const.tile([128, 512], BF16)
        nc.vector.memset(Wm[:, :], 1.0)
        Wp = psS.tile([128, XW], F32, tag="Sp")
        for w in range(36):
            nc.tensor.matmul(Wp[:, 0:512], lhsT=Wm[:, 0:128], rhs=Wm[:, :],
                             start=True, stop=True)

        m_tiles = []
        S = [dict(), dict()]
        for i, sgn in ((0, 1.0), (1, -1.0)):
            X3 = sp.tile([96, XW], F32, tag=f"X3{i}")
            nc.vector.tensor_scalar(out=X3[:, 0:CW], in0=xT3[:, 0:CW],
                                    scalar1=sgn, scalar2=0.1,
                                    op0=Alu.mult, op1=Alu.max)
            S[i]["X3"] = X3
        for i in (0, 1):
            lp3 = sp.tile([96, XW], F32, tag=f"lp3{i}")
            nc.scalar.activation(lp3[:, 0:CW], S[i]["X3"][:, 0:CW], Act.Ln)
            S[i]["lp3"] = lp3
        for i in (0, 1):
            E8 = sp.tile([96, XW], BF16, tag=f"E8{i}")
            nc.scalar.activation(E8[:, 0:CW], S[i]["lp3"][:, 0:CW], Act.Exp,
                                 bias=B8_sb[:, 0:1], scale=SIG1)
            S[i]["E8"] = E8
        for i in (0, 1):
            S8p = psS.tile([128, XW], F32, tag="Sp")
            nc.tensor.matmul(S8p[:, 0:512], lhsT=ONES_sb[:, :],
                             rhs=S[i]["E8"][:, 0:512], start=True, stop=True)
            nc.tensor.matmul(S8p[:, 512:CW], lhsT=ONES_sb[:, :],
                             rhs=S[i]["E8"][:, 512:CW], start=True, stop=True)
            S[i]["S8p"] = S8p
        for i in (0, 1):
            L8 = sp.tile([128, XW], F32, tag=f"L8{i}")
            nc.scalar.activation(L8[:, 0:CW], S[i]["S8p"][:, 0:CW], Act.Ln)
            S[i]["L8"] = L8
        for i in (0, 1):
            d8 = sp.tile([96, XW], F32, tag=f"d8{i}")
            nc.vector.scalar_tensor_tensor(out=d8[:, 0:CW], in0=S[i]["L8"][0:96, 0:CW],
                                           scalar=-1.0 / SIG1, in1=S[i]["lp3"][:, 0:CW],
                                           op0=Alu.mult, op1=Alu.add)
            S[i]["d8"] = d8
        for i in (0, 1):
            E32 = sp.tile([96, XW], BF16, tag=f"E32{i}")
            nc.scalar.activation(E32[:, 0:CW], S[i]["d8"][:, 0:CW], Act.Exp,
                                 bias=B32_sb[:, 0:1], scale=SIG2)
            S[i]["E32"] = E32
        for i in (0, 1):
            S32p = psS.tile([128, XW], F32, tag="Sp")
            nc.tensor.matmul(S32p[:, 0:512], lhsT=ONES_sb[:, :],
                             rhs=S[i]["E32"][:, 0:512], start=True, stop=True)
            nc.tensor.matmul(S32p[:, 512:CW], lhsT=ONES_sb[:, :],
                             rhs=S[i]["E32"][:, 512:CW], start=True, stop=True)
            S[i]["S32p"] = S32p
        for i in (0, 1):
            L32 = sp.tile([128, XW], F32, tag=f"L32{i}")
            nc.scalar.activation(L32[:, 0:CW], S[i]["S32p"][:, 0:CW], Act.Ln)
            S[i]["L32"] = L32
        for i in (0, 1):
            # d96 = lp - M3q + lG = d8 - L32/SIG2  (critical path, DVE)
            d96 = sp.tile([96, XW], F32, tag=f"d96{i}")
            nc.vector.scalar_tensor_tensor(out=d96[:, 0:CW], in0=S[i]["L32"][0:96, 0:CW],
                                           scalar=-1.0 / SIG2, in1=S[i]["d8"][:, 0:CW],
                                           op0=Alu.mult, op1=Alu.add)
            S[i]["d96"] = d96
        for i in (0, 1):
            E96 = sp.tile([96, XW], BF16, tag=f"E96{i}")
            nc.scalar.activation(E96[:, 0:CW], S[i]["d96"][:, 0:CW], Act.Exp,
                                 bias=B96_sb[:, 0:1], scale=T)
            S[i]["E96"] = E96
        for i in (0, 1):
            # T16 = (T/SIG1)*L8 + (T/SIG2)*L32 + CSH, off critical path (GpSimd)
            LS0 = sp.tile([128, XW], F32, tag=f"LS0{i}")
            nc.vector.scalar_tensor_tensor(out=LS0[:, 0:CW], in0=S[i]["L8"][:, 0:CW],
                                           scalar=SIG2 / SIG1, in1=S[i]["L32"][:, 0:CW],
                                           op0=Alu.mult, op1=Alu.add)
            T16e = sp.tile([128, XW], F16, tag=f"T16e{i}")
            nc.vector.tensor_scalar(out=T16e[:, 0:CW], in0=LS0[:, 0:CW],
                                    scalar1=T / SIG2, scalar2=CSH,
                                    op0=Alu.mult, op1=Alu.add)
            S[i]["T16e"] = T16e
            acc = accp.tile([128, ACW], F16, tag=f"acc{i}")
            S[i]["acc"] = acc
        for j in range(3):
            for i in (0, 1):
                Oj = psO.tile([128, XW], F32, tag="Oj")
                E96 = S[i]["E96"]
                nc.tensor.matmul(Oj[:, 0:512], lhsT=K3_sb[:, 128 * j : 128 * j + 128],
                                 rhs=E96[:, j : j + 512], start=True, stop=True)
                nc.tensor.matmul(Oj[:, 512:ACW], lhsT=K3_sb[:, 128 * j : 128 * j + 128],
                                 rhs=E96[:, j + 512 : j + ACW], start=True, stop=True)
                S[i]["Oj"] = Oj
            for i in (0, 1):
                LoS = losb.tile([128, ACW], F16, tag=f"LoS{i}")
                nc.scalar.activation(LoS[:, :], S[i]["Oj"][:, 0:ACW], Act.Ln,
                                     scale=SC_sb)
                S[i]["LoS"] = LoS
            for i in (0, 1):
                acc = S[i]["acc"]
                LoS = S[i]["LoS"]
                t16 = S[i]["T16e"]
                toff = j
                if j == 0:
                    nc.vector.tensor_tensor(
                        acc[:, 0:POSW], LoS[:, 0:POSW],
                        t16[:, toff : toff + POSW], Alu.add)
                else:
                    V = vsb.tile([128, POSW], F16, tag=f"V{i}")
                    nc.vector.tensor_tensor(
                        V[:, :], LoS[:, 0:POSW],
                        t16[:, toff : toff + POSW], Alu.add)
                    nc.vector.tensor_tensor(
                        acc[:, 0:POSW], V[:, :], acc[:, 0:POSW], Alu.max)
        for i in (0, 1):
            acc = S[i]["acc"]
            nc.vector.tensor_scalar(out=acc[:, 0:POSW], in0=acc[:, 0:POSW],
                                    scalar1=AFL_sb, scalar2=None,
                                    op0=Alu.max)
            m = msb.tile([128, HO * WO], F16, tag=f"m{i}")
            nc.scalar.activation(
                m.rearrange("q (a b) -> q a b", a=HO),
                acc.rearrange("q (a b) -> q a b", b=W)[:, :, :WO],
                Act.Exp, bias=EXM_sb, scale=1.0 / T)
            m_tiles.append(m)

        # combine + transpose into ONE psum bank, column-chunked:
        # ptall[p, 64*ci + u] = y[128*ci + p, u]
        m1, m2 = m_tiles
        ptall = psS.tile([128, XW], F32, tag="Sp")
        chunks = [(ci, min(128, HO * WO - 128 * ci)) for ci in range(8)]
        for ci, cw in chunks:
            nc.tensor.matmul(ptall[:cw, COUT * ci : COUT * ci + COUT],
                             lhsT=m1[:, 128 * ci : 128 * ci + cw],
                             rhs=MM_sb[:, 0:COUT], start=True, stop=False)
            nc.tensor.matmul(ptall[:cw, COUT * ci : COUT * ci + COUT],
                             lhsT=m2[:, 128 * ci : 128 * ci + cw],
                             rhs=MM_sb[:, COUT:128], start=False, stop=True)
        ytall = ysb.tile([128, 512], F32, tag="yt")
        nc.vector.tensor_tensor(ytall[:, :], ptall[:, 0:512], BCW_sb, Alu.add)
        nc.sync.dma_start(out=Y[:, :], in_=ytall[:, :])
    nc.finalize()
    return nc


def _host_prep(x, k1, k2, bias):
    x = np.ascontiguousarray(np.asarray(x, dtype=np.float32))
    k1 = np.asarray(k1, np.float32).reshape(3, 3, C, COUT)
    k2 = np.asarray(k2, np.float32).reshape(3, 3, C, COUT)
    Mk1 = k1.reshape(-1, COUT).max(axis=0)
    Mk2 = k2.reshape(-1, COUT).max(axis=0)
    K3 = np.zeros((96, 384), np.float32)  # cast to bf16 below
    for j in range(3):
        for g in range(3):
            K3[32 * g : 32 * g + 32, 128 * j : 128 * j + 64] = \
                np.exp(T * (k1[g, j] - Mk1))
            K3[32 * g : 32 * g + 32, 128 * j + 64 : 128 * j + 128] = \
                np.exp(T * (k2[g, j] - Mk2))
    I64 = np.eye(COUT, dtype=np.float16)
    MM = np.zeros((128, 128), np.float16)
    MM[0:64, 0:COUT] = I64
    MM[64:128, 0:COUT] = -I64
    MM[0:64, COUT:128] = -I64
    MM[64:128, COUT:128] = I64
    rng1 = Mk1 - k1.reshape(-1, COUT).min(axis=0)
    rng2 = Mk2 - k2.reshape(-1, COUT).min(axis=0)
    gc = np.minimum((16.2 + T * np.concatenate([rng1, rng2]) - 5.0) / 2.0, GCAP)
    Mk = np.concatenate([Mk1, Mk2])
    CM = np.zeros((128, 3 + 512), np.float32)
    CM[:, 0] = Mk + LG - (CSH + gc) / T
    CM[:, 1] = np.exp(gc)
    CM[:, 2] = T * (np.log(0.1) - LG) + CSH + gc
    CM[:, 3:] = np.tile(np.asarray(bias, np.float32).reshape(1, COUT), (1, 8))
    import ml_dtypes
    K3bf = K3.astype(ml_dtypes.bfloat16)
    shared = dict(K3=K3bf, MM=MM, CM=CM)
    in_maps = []
    for n in range(N_CORES):
        xT = np.zeros((C, XIN), np.float32)
        xT[:, :NPIX] = x[n].reshape(NPIX, C).T
        in_maps.append({"xT": xT, **shared})
    return in_maps


def kernel(x, k1, k2, bias):
    global last_results
    if "nc" not in _cache:
        _cache["nc"] = _build_module()
    nc = _cache["nc"]
    in_maps = _host_prep(x, k1, k2, bias)
    trace = bool(int(os.environ.get("KTRACE", "0")))
    if trace:
        _ensure_axon_ntff_hook()
    res = run_bass_kernel_spmd(
        nc, in_maps, core_ids=list(range(N_CORES)), trace=trace,
    )
    last_results = res
    # Y[p, 64*ci + u] = y[128*ci + p, u]
    ys = []
    for r in res.results:
        yd = r["Y"].reshape(128, 8, COUT).transpose(1, 0, 2).reshape(1024, COUT)
        ys.append(yd[: HO * WO].reshape(HO, WO, COUT))
    return np.stack(ys, axis=0).astype(np.float32)
